# revision 1
# baseline (speedup 1.0000x reference)
"""Trainium2 Bass kernel for nn_Detector (batched FPS detector head), v2.

Pipeline per core (256 submaps = 2 tiles of 128 submaps x 1024 points):
  1. Load pos interleaved, split channels to contiguous px/py/pz.
  2. v = px^2 + py^2 per point (exact f32, matches reference ranking).
  3. Per-submap exact 512th-smallest threshold: 19 bisection steps on the
     int32 bit pattern of v, then a top-8 window endgame (max8 of v masked
     to the remaining window + positional read of the exact k-th value).
  4. Stream-compact the 512 selected points per submap (coords as u16
     halves via local_scatter, original indices alongside).
  5. 32-point FPS vectorized across 128 submaps. Per iteration: DVE
     tensor_tensor min-update + tensor_scalar max-accumulate; the picked
     point's coords AND its original index come from a value-match
     extraction (oh = (min_d == max) on DVE; products on Pool
     tensor_tensor; sums via DVE tensor_scalar accumulate) — no DRAM
     round-trip and no argmax-index op in the chain; (c - p)^2 is one
     fused Square activation per channel.
  6. The x-row gathers and the tiny MLP (block-diagonal weights, 4 lanes)
     run pipelined inside the FPS loop per 8-pick segment; only the last
     segment trails the loop. softplus(z) = -ln(sigmoid(-z)).
     All op/engine pairings verified against the neuronx-cc ISA checks
     (Pool supports only tensor_tensor add/sub/mult; indirect-DMA offset
     APs must be [P,1] columns with flat 2-D out slices).

Outputs per core: weights [256, 32] f32, indices [256, 32] int32.
"""

import sys

for _p in ("/opt/trn_rl_repo",):
    if _p not in sys.path:
        sys.path.insert(0, _p)

import numpy as np

import concourse.bass as bass
import concourse.bacc as bacc
import concourse.mybir as mybir
from concourse.bass import IndirectOffsetOnAxis
from concourse.mybir import ActivationFunctionType as actf
from concourse.mybir import AluOpType as alu
from concourse.tile import TileContext

f32 = mybir.dt.float32
i32 = mybir.dt.int32
i16 = mybir.dt.int16
u16 = mybir.dt.uint16
u32 = mybir.dt.uint32

P = 128          # partitions = submaps per tile
NPTS = 1024      # points per submap
KPOS = 512       # closest points kept
K = 32           # FPS samples per submap
TILES = 2        # tiles per core
S_CORE = P * TILES  # submaps per core
N_CORES = 8


def build_nc():
    nc = bacc.Bacc()

    pos_in = nc.declare_dram_parameter("pos", [S_CORE * NPTS, 3], f32, isOutput=False)
    x_in = nc.declare_dram_parameter("x", [S_CORE * NPTS, 32], f32, isOutput=False)
    w1d_in = nc.declare_dram_parameter("W1d", [128, 64], f32, isOutput=False)
    w2d_in = nc.declare_dram_parameter("W2d", [64, 32], f32, isOutput=False)
    w3d_in = nc.declare_dram_parameter("W3d", [32, 4], f32, isOutput=False)
    b1d_in = nc.declare_dram_parameter("b1d", [64, 1], f32, isOutput=False)
    b2d_in = nc.declare_dram_parameter("b2d", [32, 1], f32, isOutput=False)
    b3d_in = nc.declare_dram_parameter("b3d", [4, 1], f32, isOutput=False)
    eye_in = nc.declare_dram_parameter("eye128", [128, 128], f32, isOutput=False)

    w_out = nc.declare_dram_parameter("weights_out", [S_CORE, K], f32, isOutput=True)
    i_out = nc.declare_dram_parameter("indices_out", [S_CORE, K], i32, isOutput=True)

    pos_t = pos_in[:].rearrange("(t p f) c -> t p (f c)", t=TILES, p=P)

    with TileContext(nc) as tc, tc.tile_pool(name="main", bufs=1) as pool:
        # ---- shared constants ----
        eye = pool.tile([P, 128], f32, tag="eye")
        nc.sync.dma_start(out=eye[:], in_=eye_in[:])
        rb512 = pool.tile([P, 1], i32, tag="rb512")
        nc.gpsimd.iota(rb512[:], [[1, 1]], base=0, channel_multiplier=KPOS)
        rb512f = pool.tile([P, 1], f32, tag="rb512f")
        nc.vector.tensor_copy(rb512f[:], rb512[:])

        w1d = pool.tile([P, 64], f32, tag="w1d")
        nc.sync.dma_start(out=w1d[:], in_=w1d_in[:])
        w2d = pool.tile([64, 32], f32, tag="w2d")
        nc.sync.dma_start(out=w2d[:], in_=w2d_in[:])
        w3d = pool.tile([32, 4], f32, tag="w3d")
        nc.sync.dma_start(out=w3d[:], in_=w3d_in[:])
        b1d = pool.tile([64, 1], f32, tag="b1d")
        nc.sync.dma_start(out=b1d[:], in_=b1d_in[:])
        b2d = pool.tile([32, 1], f32, tag="b2d")
        nc.sync.dma_start(out=b2d[:], in_=b2d_in[:])
        b3d = pool.tile([4, 1], f32, tag="b3d")
        nc.sync.dma_start(out=b3d[:], in_=b3d_in[:])
        b3n = pool.tile([4, 1], f32, tag="b3n")
        nc.vector.tensor_scalar_mul(b3n[:], b3d[:], -1.0)
        iota16 = pool.tile([P, NPTS], i16, tag="iota16")
        nc.gpsimd.iota(iota16[:], [[1, NPTS]], channel_multiplier=0)
        iota8 = pool.tile([P, 8], i32, tag="iota8")
        nc.gpsimd.iota(iota8[:], [[1, 8]], channel_multiplier=0)
        iota8f = pool.tile([P, 8], f32, tag="iota8f")
        nc.vector.tensor_copy(iota8f[:], iota8[:])
        zsigs = []

        # ---- load pos, split to contiguous channels, v = px^2+py^2 ----
        v, pch = [], []
        for t in range(TILES):
            pil = pool.tile([P, NPTS * 3], f32, tag=f"pos_il{t}", name=f"pos_il{t}")
            (nc.sync if t == 0 else nc.scalar).dma_start(out=pil[:], in_=pos_t[t])
            p3 = pil[:].rearrange("p (n c) -> p n c", c=3)
            chans = []
            for c, eng in ((0, nc.vector), (1, nc.gpsimd), (2, nc.scalar)):
                ch = pool.tile([P, NPTS], f32, tag=f"ch{c}_{t}", name=f"ch{c}_{t}")
                if eng is nc.scalar:
                    eng.copy(ch[:], p3[:, :, c])
                else:
                    eng.tensor_copy(ch[:], p3[:, :, c])
                chans.append(ch)
            pch.append(chans)
            sq = pool.tile([P, NPTS], f32, tag=f"sq{t}", name=f"sq{t}")
            nc.vector.tensor_tensor(sq[:], chans[0][:], chans[0][:], alu.mult)
            sqy = pool.tile([P, NPTS], f32, tag=f"sqy{t}", name=f"sqy{t}")
            nc.gpsimd.tensor_tensor(sqy[:], chans[1][:], chans[1][:], alu.mult)
            vt = pool.tile([P, NPTS], f32, tag=f"v{t}", name=f"v{t}")
            nc.vector.tensor_tensor(vt[:], sq[:], sqy[:], alu.add)
            v.append(vt)

        # ---- exact 512th-smallest v: 14 bisection steps + top-8 endgame ----
        # 14 steps on the int32 bit pattern (grid-128 midpoints are exactly
        # representable in the f32 datapath) shrink the window to a span that
        # holds <= 8 points w.h.p.  The endgame takes max8 of v masked to the
        # window and reads the exact 512th-smallest by position:
        #   tau = win_desc[cnt_le(hi) - 512]   (or lo itself if cnt_le(lo)=512)
        # Tile 0 probes on Pool (stt count), tile 1 on DVE; smalls on DVE.
        NSTEP = 16
        bst = []
        for t in range(TILES):
            b = {}
            for nm, dt_ in (("lo", f32), ("hiA", f32), ("hiB", f32), ("mid", f32),
                            ("midi", i32), ("cnt", f32), ("cond", i32),
                            ("cntlo", f32), ("cnthi", f32), ("tau", f32)):
                b[nm] = pool.tile([P, 1], dt_, tag=f"{nm}{t}", name=f"{nm}{t}")
            nc.vector.memset(b["lo"][:], 0.0)
            nc.vector.memset(b["hiA"][:], float(2 ** 31))
            nc.vector.memset(b["cntlo"][:], 0.0)
            nc.vector.memset(b["cnthi"][:], 1024.0)
            b["hi"] = [b["hiA"], b["hiB"]]
            b["hicur"] = 0
            b["junk"] = pool.tile([P, NPTS], f32, tag=f"junk{t}", name=f"junk{t}")
            bst.append(b)

        def probe(t, thr_f32_view, it=0):
            # t0 alternates ACT (Sign trick, sign-unit counts) with DVE
            # (direct count) so neither engine gates its chain; t1 on DVE.
            b = bst[t]
            if t == 0 and it % 2 == 0:
                nc.scalar.activation(
                    b["junk"][:], v[0][:], actf.Sign,
                    bias=thr_f32_view, scale=-1.0, accum_out=b["cnt"][:],
                )
                # sign units -> count units so cntlo/cnthi stay consistent
                nc.vector.tensor_scalar(
                    b["cnt"][:], b["cnt"][:], 0.5, 512.0, alu.mult, alu.add
                )
            else:
                nc.vector.tensor_scalar(
                    b["junk"][:], v[t][:], thr_f32_view, None,
                    alu.is_le, alu.add, accum_out=b["cnt"][:],
                )

        def step_tail(t):
            # cond = (#le <= 512) -> lo = mid (and cntlo = cnt)
            #                  else  hi = mid (and cnthi = cnt)
            # (select is in-place-safe only when out aliases on_false; the hi
            # update ping-pongs between two buffers instead)
            b = bst[t]
            thr = 512.0
            nc.vector.tensor_scalar(b["cond"][:], b["cnt"][:], thr, None, alu.is_le)
            nc.vector.select(b["lo"][:], b["cond"][:], b["mid"][:], b["lo"][:])
            hic = b["hi"][b["hicur"]]
            hin = b["hi"][1 - b["hicur"]]
            nc.vector.select(hin[:], b["cond"][:], hic[:], b["mid"][:])
            b["hicur"] = 1 - b["hicur"]
            nc.vector.select(b["cntlo"][:], b["cond"][:], b["cnt"][:], b["cntlo"][:])
            nc.vector.tensor_scalar(b["cond"][:], b["cnt"][:], thr, None, alu.is_gt)
            nc.vector.select(b["cnthi"][:], b["cond"][:], b["cnt"][:], b["cnthi"][:])

        def midpoint(t):
            b = bst[t]
            nc.vector.tensor_scalar(
                b["mid"][:], b["lo"][:], b["hi"][b["hicur"]][:], 0.5,
                alu.add, alu.mult,
            )

        def bisect_probe_part(t, it):
            b = bst[t]
            midpoint(t)
            nc.vector.tensor_copy(b["midi"][:], b["mid"][:])  # exact f32->i32
            probe(t, b["midi"][:].bitcast(f32), it)

        # Per step: t0's Pool probe is issued first, then t1's whole DVE
        # chain, then t0's DVE tail — so t1's probe is not queued behind t0's
        # small ops that wait on the Pool probe (in-order DVE queue).
        for it in range(NSTEP):
            bisect_probe_part(0, it)
            bisect_probe_part(1, it)
            step_tail(1)
            step_tail(0)

        # endgame per tile
        for t in range(TILES):
            b = bst[t]
            # value-space window bounds
            lobc = pool.tile([P, 1], f32, tag=f"lobc{t}", name=f"lobc{t}")
            hibc = pool.tile([P, 1], f32, tag=f"hibc{t}", name=f"hibc{t}")
            nc.vector.tensor_copy(b["midi"][:], b["lo"][:])
            nc.vector.tensor_copy(lobc[:].bitcast(i32), b["midi"][:])
            nc.vector.tensor_copy(b["midi"][:], b["hi"][b["hicur"]][:])
            nc.vector.tensor_copy(hibc[:].bitcast(i32), b["midi"][:])
            # wv = v masked to (lo, hi] (0 outside); window holds <= 8 points
            wv = b["junk"]
            wv2 = pool.tile([P, NPTS], f32, tag=f"sq{t}", name=f"wv2_{t}")
            nc.vector.tensor_scalar(wv[:], v[t][:], lobc[:], None, alu.is_gt)
            nc.vector.tensor_scalar(wv2[:], v[t][:], hibc[:], None, alu.is_le)
            nc.gpsimd.tensor_tensor(wv[:], wv[:], wv2[:], alu.mult)
            nc.gpsimd.tensor_tensor(wv[:], wv[:], v[t][:], alu.mult)
            wm8 = pool.tile([P, 8], f32, tag=f"wm8{t}", name=f"wm8{t}")
            nc.vector.max(wm8[:], wv[:])
            # idx = cnt_le(hi) - 512 (0-based position in descending order)
            idxf = pool.tile([P, 1], f32, tag=f"idxf{t}", name=f"idxf{t}")
            nc.vector.tensor_scalar(
                idxf[:], b["cnthi"][:], -512.0, None, alu.add
            )
            oh8 = pool.tile([P, 8], f32, tag=f"oh8{t}", name=f"oh8{t}")
            nc.vector.tensor_scalar(oh8[:], iota8f[:], idxf[:], None, alu.is_equal)
            j8 = pool.tile([P, 8], f32, tag=f"j8{t}", name=f"j8{t}")
            nc.vector.tensor_tensor(j8[:], oh8[:], wm8[:], alu.mult)
            nc.vector.tensor_scalar(
                j8[:], j8[:], 0.0, None, alu.add, alu.add, accum_out=b["tau"][:]
            )
            # cnt_le(lo) == 512 -> tau = lo's value itself
            nc.vector.tensor_scalar(
                b["cond"][:], b["cntlo"][:], 512.0, None, alu.is_equal
            )
            nc.vector.copy_predicated(b["tau"][:], b["cond"][:], lobc[:])

        tau = [bst[t]["tau"] for t in range(TILES)]  # exact 512-smallest thr

        # ---- compact: mask -> ranks -> scatter coords/indices ----
        pcc, selc = [], []
        for t in range(TILES):
            tau_f = tau[t][:]
            mask = pool.tile([P, NPTS], f32, tag=f"mask{t}", name=f"mask{t}")
            nc.vector.tensor_scalar(mask[:], v[t][:], tau_f, None, alu.is_le)
            rank = pool.tile([P, NPTS], f32, tag=f"rank{t}", name=f"rank{t}")
            nc.vector.tensor_tensor_scan(
                rank[:], mask[:], mask[:], 0.0, alu.add, alu.bypass
            )
            # slot = rank * mask - 1  (-1 for unselected -> ignored by scatter)
            slot = pool.tile([P, NPTS], f32, tag=f"slot{t}", name=f"slot{t}")
            nc.vector.tensor_tensor(slot[:], rank[:], mask[:], alu.mult)
            nc.vector.tensor_scalar(slot[:], slot[:], -1.0, None, alu.add)

            # u16-half slot indices: even half -> 2*slot, odd half -> 2*slot+1
            idx2f = pool.tile([P, 2 * NPTS], f32, tag=f"pos_il{t}", name=f"idx2f{t}")
            i2v = idx2f[:].rearrange("p (n two) -> p n two", two=2)
            nc.vector.tensor_scalar_mul(i2v[:, :, 0], slot[:], 2.0)
            nc.vector.tensor_scalar(i2v[:, :, 1], slot[:], 2.0, 1.0, alu.mult, alu.add)
            idx2 = pool.tile([P, 2 * NPTS], i16, tag=f"idx2_{t}", name=f"idx2_{t}")
            nc.vector.tensor_copy(idx2[:], idx2f[:])

            # compacted coord channels (as u16 halves of f32)
            chc = []
            for c in range(3):
                cc = pool.tile([P, 2 * KPOS], u16, tag=f"cc{c}_{t}", name=f"cc{c}_{t}")
                nc.gpsimd.local_scatter(
                    cc[:], pch[t][c][:].bitcast(u16), idx2[:],
                    channels=P, num_elems=2 * KPOS, num_idxs=2 * NPTS,
                )
                chc.append(cc)
            pcc.append([cc[:].bitcast(f32) for cc in chc])

            # compacted original indices (i16 scatter of iota, then widen)
            slot16 = pool.tile([P, NPTS], i16, tag=f"slot16{t}", name=f"slot16{t}")
            nc.vector.tensor_copy(slot16[:], slot[:])
            sel16 = pool.tile([P, KPOS], i16, tag=f"sel16{t}", name=f"sel16{t}")
            nc.gpsimd.local_scatter(
                sel16[:], iota16[:], slot16[:],
                channels=P, num_elems=KPOS, num_idxs=NPTS,
            )
            self_f = pool.tile([P, KPOS], f32, tag=f"selff{t}", name=f"selff{t}")
            nc.vector.tensor_copy(self_f[:], sel16[:])
            selc.append(self_f)

        # ---- FPS state ----
        st = []
        for t in range(TILES):
            s = {}
            s["pxc"], s["pyc"], s["pzc"] = pcc[t]
            s["m8"] = pool.tile([P, 8], f32, tag=f"m8_{t}", name=f"m8_{t}")
            nc.vector.memset(s["m8"][:], -1.0)
            s["pidx0"] = pool.tile([P, 8], u32, tag=f"pidx0_{t}", name=f"pidx0_{t}")
            s["m80"] = pool.tile([P, 8], f32, tag=f"m80_{t}", name=f"m80_{t}")
            s["ogmat"] = pool.tile([P, K], f32, tag=f"ogmat_{t}", name=f"ogmat_{t}")
            s["oh"] = pool.tile([P, KPOS], f32, tag=f"oh_{t}", name=f"oh_{t}")
            s["minv"] = pool.tile([P, 1], f32, tag=f"minv_{t}", name=f"minv_{t}")
            s["cx"] = pool.tile([P, 1], f32, tag=f"cx_{t}", name=f"cx_{t}")
            s["cy"] = pool.tile([P, 1], f32, tag=f"cy_{t}", name=f"cy_{t}")
            s["cz"] = pool.tile([P, 1], f32, tag=f"cz_{t}", name=f"cz_{t}")
            s["jx"] = pool.tile([P, KPOS], f32, tag=f"rank{t}", name=f"jx_{t}")
            s["jy"] = pool.tile([P, KPOS], f32, tag=f"slot{t}", name=f"jy_{t}")
            s["jz"] = pool.tile([P, KPOS], f32, tag=f"idx2_{t}", name=f"jz_{t}")
            s["jog"] = pool.tile([P, KPOS], f32, tag=f"jog_{t}", name=f"jog_{t}")
            s["sqx"] = pool.tile([P, KPOS], f32, tag=f"sq{t}", name=f"sqx_{t}")
            s["sqy"] = pool.tile([P, KPOS], f32, tag=f"sqy{t}", name=f"sqys_{t}")
            s["sqz"] = pool.tile([P, KPOS], f32, tag=f"mask{t}", name=f"sqzs_{t}")
            s["s1"] = pool.tile([P, KPOS], f32, tag=f"slot16{t}", name=f"s1_{t}")
            # d2 / md buffers are created lazily at first write: they re-tag
            # the full-channel slots (ch0/ch1/ch2) which the FPS init still
            # reads — allocating them here would deadlock the tile pool.
            s["d2"] = None
            s["md"] = [None, None]
            xrow = pool.tile([P, 1], i32, tag=f"xrow_{t}", name=f"xrow_{t}")
            nc.gpsimd.iota(
                xrow[:], [[1, 1]], base=t * P * NPTS, channel_multiplier=NPTS
            )
            xrowf = pool.tile([P, 1], f32, tag=f"xrowf_{t}", name=f"xrowf_{t}")
            nc.vector.tensor_copy(xrowf[:], xrow[:])
            s["xrowf"] = xrowf
            # per-segment MLP tiles (reused across the 4 segments of 8 picks)
            s["growf8"] = pool.tile([P, 8], f32, tag=f"growf8_{t}", name=f"growf8_{t}")
            s["grow8"] = pool.tile([P, 8], i32, tag=f"grow8_{t}", name=f"grow8_{t}")
            s["xg8"] = pool.tile([P, 256], f32, tag=f"xg8_{t}", name=f"xg8_{t}")
            s["h1seg"] = pool.tile([64, 256], f32, tag=f"h1s_{t}", name=f"h1s_{t}")
            s["h2seg"] = pool.tile([32, 256], f32, tag=f"h2s_{t}", name=f"h2s_{t}")
            s["s4seg"] = pool.tile([4, 256], f32, tag=f"s4s_{t}", name=f"s4s_{t}")
            s["zsig"] = pool.tile([4, 1024], f32, tag=f"zsig_{t}", name=f"zsig_{t}")
            s["orig32"] = pool.tile([P, K], i32, tag=f"orig32_{t}", name=f"orig32_{t}")
            s["wout"] = pool.tile([P, K], f32, tag=f"wout_{t}", name=f"wout_{t}")
            s["xt4seg"] = None  # lazily tagged over junk{t} after init uses it
            st.append(s)

        import contextlib

        _ps_stack = contextlib.ExitStack()
        psp = [
            _ps_stack.enter_context(
                tc.tile_pool(name=f"psum{t}", bufs=1, space="PSUM")
            )
            for t in range(TILES)
        ]

        # segments: three 8-pick segments pipelined in-loop, then two
        # 4-pick mini-segments so most of the tail MLP also overlaps the
        # last FPS iterations (grp-aligned: picks 24-27 = grp 6, 28-31 = 7)
        SEGS = [(0, 8), (8, 16), (16, 24), (24, 28), (28, 32)]

        def seg_max_stage(lo, hi):
            return 6 if hi - lo == 8 else 4

        def emit_seg(t, si, stage):
            s = st[t]
            lo, hi = SEGS[si]
            w = hi - lo
            sl = slice(lo, hi)
            if w == 8 and stage == 0:
                return
            if (w == 8 and stage == 1) or (w == 4 and stage == 0):
                nc.vector.tensor_copy(s["orig32"][:, sl], s["ogmat"][:, sl])
                nc.vector.tensor_scalar(
                    s["growf8"][:, 0:w], s["ogmat"][:, sl], s["xrowf"][:],
                    None, alu.add,
                )
                nc.vector.tensor_copy(s["grow8"][:, 0:w], s["growf8"][:, 0:w])
                for j in range(4):
                    nc.gpsimd.indirect_dma_start(
                        out=s["xg8"][:, j * 32 : (j + 1) * 32],
                        out_offset=None,
                        in_=x_in[:],
                        in_offset=IndirectOffsetOnAxis(
                            ap=s["grow8"][:, j : j + 1], axis=0
                        ),
                    )
                return
            if w == 8 and stage == 2:
                for j in range(4, 8):
                    nc.gpsimd.indirect_dma_start(
                        out=s["xg8"][:, j * 32 : (j + 1) * 32],
                        out_offset=None,
                        in_=x_in[:],
                        in_offset=IndirectOffsetOnAxis(
                            ap=s["grow8"][:, j : j + 1], axis=0
                        ),
                    )
            elif (w == 8 and stage == 3) or (w == 4 and stage == 1):
                xg3 = s["xg8"][:].rearrange("p (k f) -> p k f", f=32)
                ps_xt = psp[t].tile([P, w * 32], f32, tag=f"ps_xt{t}")
                s["ps_xt"] = ps_xt
                for j in range(w):
                    g = lo + j
                    lane = g % 4
                    grp2 = (g // 4) - (lo // 4)
                    nc.tensor.matmul(
                        ps_xt[lane * 32 : (lane + 1) * 32,
                              grp2 * 128 : (grp2 + 1) * 128],
                        xg3[:, j, :],
                        eye[:],
                        tile_position=(0, lane * 32),
                    )
            elif (w == 8 and stage == 4) or (w == 4 and stage == 2):
                if s["xt4seg"] is None:
                    s["xt4seg"] = pool.tile(
                        [P, 256], f32, tag=f"junk{t}", name=f"xt4s_{t}"
                    )
                nc.scalar.copy(s["xt4seg"][:, 0 : w * 32], s["ps_xt"][:])
                ps_h = psp[t].tile([64, w * 32], f32, tag=f"ps_h{t}")
                s["ps_h"] = ps_h
                nc.tensor.matmul(ps_h[:, :], w1d[:], s["xt4seg"][:, 0 : w * 32])
                nc.scalar.activation(
                    s["h1seg"][:, 0 : w * 32], ps_h[:], actf.Relu,
                    bias=b1d[:], scale=1.0,
                )
            elif (w == 8 and stage == 5) or (w == 4 and stage == 3):
                ps_h2 = psp[t].tile([32, w * 32], f32, tag=f"ps_h2{t}")
                nc.tensor.matmul(ps_h2[:, :], w2d[:], s["h1seg"][:, 0 : w * 32])
                nc.scalar.activation(
                    s["h2seg"][:, 0 : w * 32], ps_h2[:], actf.Relu,
                    bias=b2d[:], scale=1.0,
                )
                ps_z = psp[t].tile([4, w * 32], f32, tag=f"ps_h{t}")
                nc.tensor.matmul(ps_z[:, :], w3d[:], s["h2seg"][:, 0 : w * 32])
                nc.scalar.activation(
                    s["zsig"][:, lo * 32 : hi * 32], ps_z[:],
                    actf.Sigmoid, bias=b3n[:], scale=-1.0,
                )
            elif (w == 8 and stage == 6) or (w == 4 and stage == 4):
                nc.scalar.activation(
                    s["s4seg"][:, 0 : w * 32],
                    s["zsig"][:, lo * 32 : hi * 32], actf.Ln,
                )
                ps_w = psp[t].tile([P, w], f32, tag=f"ps_xt{t}")
                for c2 in range(w // 4):
                    nc.tensor.transpose(
                        ps_w[:, c2 * 4 : (c2 + 1) * 4],
                        s["s4seg"][:, c2 * 128 : (c2 + 1) * 128],
                        eye[0:4, 0:4],
                    )
                nc.vector.tensor_scalar_mul(s["wout"][:, sl], ps_w[:], -1.0)

        def ex_chans(s, t, k, coords):
            # value-match extraction channels: original index always; the
            # coordinate channels only while further distances are needed
            chans = [(selc[t][:], "jog", s["ogmat"][:, k : k + 1])]
            if coords:
                chans = [
                    (s["pxc"], "jx", s["cx"][:]),
                    (s["pyc"], "jy", s["cy"][:]),
                    (s["pzc"], "jz", s["cz"][:]),
                ] + chans
            return chans

        def ex_mask(s, mask_src, mx_view):
            # oh = (min_d == mx) on DVE
            nc.vector.tensor_scalar(
                s["oh"][:], mask_src, mx_view, None, alu.is_equal
            )

        def ex_products(s, chans):
            for (ch, cj, _acc) in chans:
                nc.gpsimd.tensor_tensor(s[cj][:], ch, s["oh"][:], alu.mult)

        def ex_accums(s, chans):
            for (_ch, cj, acc) in chans:
                nc.vector.tensor_scalar(
                    s[cj][:], s[cj][:], 0.0, None, alu.add, alu.add,
                    accum_out=acc,
                )

        def squares(s):
            # exact (c - p)^2 per channel, fused on ACT
            nc.scalar.activation(
                s["sqx"][:], s["pxc"], actf.Square, bias=s["cx"][:], scale=-1.0
            )
            nc.scalar.activation(
                s["sqy"][:], s["pyc"], actf.Square, bias=s["cy"][:], scale=-1.0
            )
            nc.scalar.activation(
                s["sqz"][:], s["pzc"], actf.Square, bias=s["cz"][:], scale=-1.0
            )

        def sums(s, out_ap, t=0):
            # reference associativity: (dx^2 + dy^2) + dz^2; engines split
            # per tile to balance Pool vs DVE load
            nc.vector.tensor_tensor(s["s1"][:], s["sqx"][:], s["sqy"][:], alu.add)
            nc.gpsimd.tensor_tensor(out_ap, s["s1"][:], s["sqz"][:], alu.add)

        # ---- FPS init: pick 0 = argmin v (original index); md0 = d2 to it ----
        for t in range(TILES):
            s = st[t]
            vneg = pool.tile([P, NPTS], f32, tag=f"junk{t}", name=f"vneg_{t}")
            nc.vector.tensor_scalar_mul(vneg[:], v[t][:], -1.0)
            nc.vector.max(s["m80"][:], vneg[:])
            nc.vector.max_index(s["pidx0"][:], s["m80"][:], vneg[:])
            nc.vector.tensor_copy(s["ogmat"][:, 0:1], s["pidx0"][:, 0:1])
            nc.vector.tensor_scalar_mul(s["minv"][:], s["m80"][:, 0:1], -1.0)
            # c0 via value-match on the FULL arrays (v == min v)
            oh0 = pool.tile([P, NPTS], f32, tag=f"sqy{t}", name=f"oh0_{t}")
            nc.vector.tensor_scalar(
                oh0[:], v[t][:], s["minv"][:], None, alu.is_equal
            )
            for (chf, cc) in (
                (pch[t][0], "cx"), (pch[t][1], "cy"), (pch[t][2], "cz"),
            ):
                nc.gpsimd.tensor_tensor(vneg[:], oh0[:], chf[:], alu.mult)
                nc.vector.tensor_scalar(
                    vneg[:], vneg[:], 0.0, None, alu.add, alu.add,
                    accum_out=s[cc][:],
                )
            squares(s)
            s["md"][0] = pool.tile([P, KPOS], f32, tag=f"ch1_{t}", name=f"mdA_{t}")
            sums(s, s["md"][0][:])

        # ---- FPS loop ----
        # Emission order is the per-engine schedule (in-order queues): the
        # update/argmax + extracts + squares come per tile, but both tiles'
        # s1/d2 sums are emitted after both tile blocks so the next ttr is
        # not head-of-line blocked behind a sum waiting on the ACT squares.
        curbuf = [0 for _ in range(TILES)]
        for k in range(1, K):
            chansk = []
            for t in range(TILES):
                s = st[t]
                cur = s["md"][curbuf[t]]
                if k == 1:
                    nc.vector.tensor_scalar(
                        s["jog"][:], cur[:], 0.0, None, alu.add, alu.max,
                        accum_out=s["m8"][:, 0:1],
                    )
                else:
                    if s["md"][1 - curbuf[t]] is None:
                        s["md"][1 - curbuf[t]] = pool.tile(
                            [P, KPOS], f32, tag=f"ch2_{t}", name=f"mdB_{t}"
                        )
                    nxt = s["md"][1 - curbuf[t]]
                    # min-update on DVE (Pool tensor_tensor only supports
                    # add/sub/mult on HW), then the max via a DVE
                    # tensor_scalar max-accumulate
                    nc.vector.tensor_tensor(nxt[:], cur[:], s["d2"][:], alu.min)
                    nc.vector.tensor_scalar(
                        s["jog"][:], nxt[:], 0.0, None, alu.add, alu.max,
                        accum_out=s["m8"][:, 0:1],
                    )
                    curbuf[t] = 1 - curbuf[t]
                    cur = nxt
                ex_mask(s, cur[:], s["m8"][:, 0:1])
                chansk.append(ex_chans(s, t, k, coords=(k < K - 1)))
            for t in range(TILES):
                ex_products(st[t], chansk[t])
            for t in range(TILES):
                ex_accums(st[t], chansk[t])
            if k < K - 1:
                for t in range(TILES):
                    squares(st[t])
                for t in range(TILES):
                    s = st[t]
                    if s["d2"] is None:
                        s["d2"] = pool.tile(
                            [P, KPOS], f32, tag=f"ch0_{t}", name=f"d2_{t}"
                        )
                    sums(s, s["d2"][:], t)
            # staged segment MLP work for completed pick-segments
            for t in range(TILES):
                for si, (lo, hi) in enumerate(SEGS):
                    stage = k - (hi - 1) - 2 * t
                    if 0 <= stage <= seg_max_stage(lo, hi):
                        emit_seg(t, si, stage)

        # ---- remaining (post-loop) segment stages + final output DMAs ----
        for si, (lo, hi) in enumerate(SEGS):
            for t in range(TILES):
                first_post = max(0, 31 - (hi - 1) - 2 * t + 1)
                for stage in range(first_post, seg_max_stage(lo, hi) + 1):
                    emit_seg(t, si, stage)
        _ps_stack.close()
        for t in range(TILES):
            s = st[t]
            nc.sync.dma_start(out=i_out[t * P : (t + 1) * P, :], in_=s["orig32"][:])
            nc.sync.dma_start(out=w_out[t * P : (t + 1) * P, :], in_=s["wout"][:])

    nc.compile()
    return nc


def _host_prep(W1, b1, W2, b2, W3, b3):
    """Block-diagonal 4-lane weight stacks + replicated biases."""
    W1 = np.asarray(W1, np.float32)
    W2 = np.asarray(W2, np.float32)
    W3 = np.asarray(W3, np.float32)
    W1d = np.zeros((128, 64), np.float32)
    W2d = np.zeros((64, 32), np.float32)
    W3d = np.zeros((32, 4), np.float32)
    for l in range(4):
        W1d[l * 32 : (l + 1) * 32, l * 16 : (l + 1) * 16] = W1
        W2d[l * 16 : (l + 1) * 16, l * 8 : (l + 1) * 8] = W2
        W3d[l * 8 : (l + 1) * 8, l : l + 1] = W3
    b1d = np.tile(np.asarray(b1, np.float32), 4).reshape(64, 1)
    b2d = np.tile(np.asarray(b2, np.float32), 4).reshape(32, 1)
    b3d = np.tile(np.asarray(b3, np.float32), 4).reshape(4, 1)
    return W1d, W2d, W3d, b1d, b2d, b3d


_NC = None


def _get_nc():
    global _NC
    if _NC is None:
        _NC = build_nc()
    return _NC


def kernel(x, pos, batch, W1, b1, W2, b2, W3, b3):
    from concourse.bass_utils import run_bass_kernel_spmd

    x = np.ascontiguousarray(np.asarray(x, np.float32))
    pos = np.ascontiguousarray(np.asarray(pos, np.float32))
    W1d, W2d, W3d, b1d, b2d, b3d = _host_prep(W1, b1, W2, b2, W3, b3)
    eye128 = np.eye(128, dtype=np.float32)

    rows = S_CORE * NPTS
    in_maps = []
    for c in range(N_CORES):
        in_maps.append(
            {
                "pos": pos[c * rows : (c + 1) * rows],
                "x": x[c * rows : (c + 1) * rows],
                "W1d": W1d, "W2d": W2d, "W3d": W3d,
                "b1d": b1d, "b2d": b2d, "b3d": b3d,
                "eye128": eye128,
            }
        )

    nc = _get_nc()
    res = run_bass_kernel_spmd(nc, in_maps, list(range(N_CORES))).results
    weights = np.concatenate([res[c]["weights_out"] for c in range(N_CORES)], axis=0)
    indices = np.concatenate(
        [res[c]["indices_out"].astype(np.int32) for c in range(N_CORES)], axis=0
    )
    return weights, indices

